# revision 47
# baseline (speedup 1.0000x reference)
"""ChebyKAN layer on 8 Trainium2 NeuronCores.

y = einsum('dbi,dio->bo', cheby_basis(tanh(x)), cheby_coeffs)

Strategy (per core, data-parallel over batch):
  - each core takes 1024 rows of x (8192/8) and the full coeffs
  - host preprocessing: x pre-transposed to [i, b]; cheby_coeffs for
    degrees 1..8 cast to bf16 in [d, i_tile, p, o] layout; the degree-0
    term (T_0 == 1) folded into a per-output bias computed on host
  - tanh on the scalar engine (both a bf16 copy for the matmul feed and
    an fp32 copy for the recurrence); Chebyshev recurrence on the vector
    engine with bf16 state (the 2*xt*T product uses the fp32 xt, the
    subtract writes bf16 directly), so no separate cast ops exist and
    the scalar queue stays short (rel err ~3e-3, verified vs fp64)
  - contraction as bf16 matmuls (full-rate, fp32 PSUM accumulation):
    stationary = W[d, i-tile, o-tile] (bf16 -> fast weight load),
    moving = T_d[i-tile, b-half], psum [o-tile 128, b-half 512] x 8 banks
  - W streamed from HBM once per half directly as bf16 (no on-device
    cast); two b-halves of 512
  - last degree runs o-tile-major so banks close early; evacuation adds
    the bias (scalar activation / vector tensor_scalar with per-partition
    operand) with 8 in-flight buffers so y DMAs overlap the final MMs
  - a few dummy matmuls at t=0 lift the HAM clock gate before real work
  - output is y.T per core; host transposes and concatenates
"""

import numpy as np
import ml_dtypes

import concourse.bass as bass
import concourse.tile as tile
from concourse import bacc, mybir
from concourse import bass_utils

N_CORES = 8
B = 8192
IC = 1024
OC = 1024
DEG = 8  # degrees 1..8 on device; degree 0 folded into bias
BC = B // N_CORES  # 1024 batch rows per core
P = 128
NI = IC // P  # 8 i-tiles
NO = OC // P  # 8 o-tiles
BH = BC // 2  # 512, b-half
F32 = mybir.dt.float32
BF16 = mybir.dt.bfloat16

# W slab granularity (in i-tiles): first-degree slabs of half 0 are small
# so the first matmul's W-DMA is short.
_D1_SLABS = [1, 1, 2, 2, 2]
_D_SLABS = [2, 2, 2, 2]
# x DMA chunk granularity (in i-tiles). Each DMA trigger costs ~0.7us of
# issuing-engine queue time, so few chunks; first chunk small for latency.
_X_CHUNKS_H0 = [1, 1, 2, 4]
_X_CHUNKS_H1 = [4, 4]
N_WARM = 16  # dummy matmuls at t=0: bridge gaplessly into the real
# stream AND delay it slightly (~13.5us) so the x pipeline banks ahead
# of degree-1 consumption; the HAM clock gate opens during the bridge


def _build(tanh_scale: float, tanh_bias: float):
    nc = bacc.Bacc("TRN2", target_bir_lowering=False, debug=False, num_devices=N_CORES)

    # x is shipped in SBUF layout ([2*128, il*512+b], transposed on host)
    # so its DMA is a pure contiguous copy with wide descriptor lines; y
    # is half-major ([2*1024, 512]) so each store is a contiguous block
    xT_d = nc.dram_tensor("xT", [2 * P, NI * BH], F32, kind="ExternalInput").ap()
    w_d = nc.dram_tensor("w", [P, DEG * NI * OC], BF16, kind="ExternalInput").ap()
    bias_d = nc.dram_tensor("bias", [P, NO], F32, kind="ExternalInput").ap()
    yt_d = nc.dram_tensor("yt", [2 * OC, BH], F32, kind="ExternalOutput").ap()

    from concourse.alu_op_type import AluOpType

    TANH = mybir.ActivationFunctionType.Tanh
    IDENT = mybir.ActivationFunctionType.Identity

    with tile.TileContext(nc) as tc:
        with (
            tc.tile_pool(name="const", bufs=1) as constp,
            tc.tile_pool(name="xin", bufs=5) as xinp,
            tc.tile_pool(name="xt", bufs=2) as xtp,
            tc.tile_pool(name="tb", bufs=6) as tbp,
            tc.tile_pool(name="prod", bufs=2) as prodp,
            tc.tile_pool(name="wst", bufs=8) as wp,
            tc.tile_pool(name="evac", bufs=8) as evacp,
            tc.tile_pool(name="ps", bufs=8, space=bass.MemorySpace.PSUM) as psp,
        ):
            bias_s = constp.tile([P, NO], F32)

            # PE warm-up: garbage matmuls so the HAM clock gate opens
            # before the first real matmul arrives (~3.4us of activity).
            warm = constp.tile([P, BH], BF16)
            nc.gpsimd.memset(warm[:], 1.0)
            warm_ps = psp.tile([P, BH], F32, tag="ps", name="warm_ps")
            for _ in range(N_WARM):
                nc.tensor.matmul(
                    warm_ps[:], warm[:, 0:P], warm[:], start=True, stop=True
                )

            def emit_w_slabs(h, d, slab_sizes, it0=0):
                """DMA W[d] i-tile slabs (bf16, no cast needed)."""
                out = []
                for ws, nt in enumerate(slab_sizes):
                    wst = wp.tile(
                        [P, nt * OC], BF16, tag="wst", name=f"w_{h}_{d}_{ws}"
                    )
                    c0 = ((d - 1) * NI + it0) * OC
                    nc.sync.dma_start(wst[:], w_d[:, c0 : c0 + nt * OC])
                    out.append((it0, nt, wst))
                    it0 += nt
                return out

            def emit_x_dma(h, ci, it0, nt, ring):
                """One x chunk DMA trigger on the chosen HWDGE ring."""
                xst = xinp.tile([P, nt * BH], F32, tag="xin", name=f"xs_{h}_{ci}")
                eng = nc.scalar if ring == "scalar" else nc.sync
                eng.dma_start(
                    xst[:],
                    xT_d[h * P : (h + 1) * P, it0 * BH : (it0 + nt) * BH],
                )
                return (it0, nt, xst)

            def emit_x_tanh(h, xsts):
                """tanh -> bf16 xtb (T_1 moving, critical path) then
                tanh -> fp32 xt (recurrence input, off critical path)."""
                xt = xtp.tile([P, NI * BH], F32, tag="xt", name=f"xt_{h}")
                xtb = tbp.tile([P, NI * BH], BF16, tag="tb", name=f"xtb_{h}")
                for it0, nt, xst in xsts:
                    sl = slice(it0 * BH, (it0 + nt) * BH)
                    nc.scalar.activation(
                        xtb[:, sl], xst[:], TANH, bias=tanh_bias, scale=tanh_scale
                    )
                for it0, nt, xst in xsts:
                    sl = slice(it0 * BH, (it0 + nt) * BH)
                    nc.scalar.activation(
                        xt[:, sl], xst[:], TANH, bias=tanh_bias, scale=tanh_scale
                    )
                return xt, xtb

            def emit_xt(h, chunks):
                # stripe chunks across both HWDGE rings so x transfers
                # run in parallel instead of serializing on one ring
                xsts = []
                it0 = 0
                for ci, nt in enumerate(chunks):
                    ring = "scalar" if ci % 2 == 0 else "sync"
                    xsts.append(emit_x_dma(h, ci, it0, nt, ring))
                    it0 += nt
                xt, xtb = emit_x_tanh(h, xsts)
                return xt, xtb

            def emit_state(h, d, xt, t_m1, t_m2):
                """T_d = 2*xt*T_{d-1} - T_{d-2}; product in fp32 (xt is
                fp32), subtract writes the bf16 state directly."""
                tb_new = tbp.tile([P, NI * BH], BF16, tag="tb", name=f"tb_{h}_{d}")
                QS = NI * BH // 4
                for q in range(4):
                    sl = slice(q * QS, (q + 1) * QS)
                    prod = prodp.tile([P, QS], F32, tag="prod", name=f"pr_{h}_{d}_{q}")
                    nc.vector.scalar_tensor_tensor(
                        prod[:], xt[:, sl], 2.0, t_m1[:, sl],
                        AluOpType.mult, AluOpType.mult,
                    )
                    if d == 2:
                        # T2 = 2*xt^2 - 1
                        nc.vector.tensor_scalar_sub(tb_new[:, sl], prod[:], 1.0)
                    else:
                        nc.vector.tensor_sub(tb_new[:, sl], prod[:], t_m2[:, sl])
                return tb_new

            xts = [None, None]
            xtbs = [None, None]

            for h in range(2):
                if h == 0:
                    xts[0], xtbs[0] = emit_xt(0, _X_CHUNKS_H0)
                    w1_slabs = emit_w_slabs(0, 1, _D1_SLABS)
                    nc.sync.dma_start(bias_s[:], bias_d)
                xt, xtb = xts[h], xtbs[h]
                accs = [
                    psp.tile([P, BH], F32, tag="ps", name=f"acc_h{h}_o{ot}")
                    for ot in range(NO)
                ]

                t_m1 = xt  # T_{d-1}: fp32 xt for d=2, bf16 state after
                t_m2 = None  # T_{d-2}: fp32 xt for d=3, bf16 state after
                for d in range(1, DEG + 1):
                    if d == 1:
                        tb_d = xtb
                    else:
                        tb_d = emit_state(h, d, xt, t_m1, t_m2)
                        t_m2 = xt if d == 2 else t_m1
                        t_m1 = tb_d

                    if h == 0 and d == 1:
                        wr_slabs = w1_slabs
                    else:
                        wr_slabs = emit_w_slabs(h, d, _D_SLABS)

                    if d < DEG:
                        for it0, nt, wst in wr_slabs:
                            for il in range(nt):
                                it = it0 + il
                                rhs = tb_d[:, it * BH : (it + 1) * BH]
                                for ot in range(NO):
                                    nc.tensor.matmul(
                                        accs[ot][:],
                                        wst[:, il * OC + ot * P : il * OC + (ot + 1) * P],
                                        rhs,
                                        start=(d == 1 and it == 0),
                                        stop=False,
                                    )
                    else:
                        # last degree ot-major: each bank closes early, its
                        # evac + y DMA overlap the remaining matmuls
                        wlist = []
                        for it0, nt, wst in wr_slabs:
                            for il in range(nt):
                                wlist.append((wst, il))
                        for ot in range(NO):
                            for it in range(NI):
                                wst, il = wlist[it]
                                rhs = tb_d[:, it * BH : (it + 1) * BH]
                                nc.tensor.matmul(
                                    accs[ot][:],
                                    wst[:, il * OC + ot * P : il * OC + (ot + 1) * P],
                                    rhs,
                                    start=False,
                                    stop=(it == NI - 1),
                                )
                            ev = evacp.tile([P, BH], F32, tag="evac", name=f"ev{h}_{ot}")
                            bias_ap = bias_s[:, ot : ot + 1]
                            if ot % 2 == 0:
                                nc.vector.tensor_scalar_add(ev[:], accs[ot][:], bias_ap)
                            else:
                                nc.scalar.activation(ev[:], accs[ot][:], IDENT, bias=bias_ap)
                            # odd banks (incl. the final one) trigger from
                            # the sync queue so the last y DMA is not
                            # serialized behind the scalar evacuation
                            yeng = nc.sync if ot % 2 == 1 else nc.scalar
                            yeng.dma_start(
                                yt_d[h * OC + ot * P : h * OC + (ot + 1) * P, :],
                                ev[:],
                            )

                    if h == 0 and d == 2:
                        xts[1], xtbs[1] = emit_xt(1, _X_CHUNKS_H1)

    nc.compile()
    return nc


_CACHE: dict = {}


def make_in_maps(x, w):
    # W in SBUF layout: [p, (d, il, o)] so each slab DMA is contiguous
    wb = np.ascontiguousarray(
        w[1:]
        .astype(ml_dtypes.bfloat16)
        .reshape(DEG, NI, P, OC)
        .transpose(2, 0, 1, 3)
        .reshape(P, DEG * NI * OC)
    )
    bias = w[0].astype(np.float64).sum(axis=0).astype(np.float32)
    bias_t = np.ascontiguousarray(bias.reshape(NO, P).T)
    in_maps = []
    for c in range(N_CORES):
        halves = []
        for h in range(2):
            xs = x[c * BC + h * BH : c * BC + (h + 1) * BH]  # [BH, IC]
            halves.append(
                xs.reshape(BH, NI, P).transpose(2, 1, 0).reshape(P, NI * BH)
            )
        xT = np.ascontiguousarray(np.concatenate(halves, axis=0))
        in_maps.append({"xT": xT, "w": wb, "bias": bias_t})
    return in_maps


def kernel(x, cheby_coeffs, tanh_scale, tanh_bias):
    x = np.ascontiguousarray(np.asarray(x, dtype=np.float32))
    w = np.ascontiguousarray(np.asarray(cheby_coeffs, dtype=np.float32))
    ts = float(np.asarray(tanh_scale))
    tb = float(np.asarray(tanh_bias))

    key = (ts, tb)
    if key not in _CACHE:
        _CACHE[key] = _build(ts, tb)
    nc = _CACHE[key]

    in_maps = make_in_maps(x, w)
    res = bass_utils.run_bass_kernel_spmd(
        nc, in_maps, core_ids=list(range(N_CORES)), trace=False
    )

    y = np.empty((B, OC), dtype=np.float32)
    for c in range(N_CORES):
        yt = res.results[c]["yt"]  # [2*OC, BH], half-major
        y[c * BC : c * BC + BH, :] = yt[:OC].T
        y[c * BC + BH : (c + 1) * BC, :] = yt[OC:].T
    return y
